# revision 2
# baseline (speedup 1.0000x reference)
import sys
import numpy as np
import ml_dtypes

sys.path.insert(0, '/opt/trn_rl_repo')

import concourse.bacc as bacc
import concourse.mybir as mybir
from concourse.bass_utils import run_bass_kernel_spmd
from concourse.tile import TileContext
from contextlib import ExitStack
from collections import deque

f32 = mybir.dt.float32
f32r = mybir.dt.float32r
bf16 = mybir.dt.bfloat16
AF = mybir.ActivationFunctionType
ALU = mybir.AluOpType

D_MODEL = 1024
N_HEAD = 16
D_HEAD = 64
B = 4
T = 2048
N_CORES = 8
HPC = N_HEAD // 2        # 8 heads per core
HD = HPC * D_HEAD        # 512 head-dims per core
NTK = D_MODEL // 128     # 8 k-chunks over model dim
NTT = T // 128           # 16 T-tiles of 128
NJC = T // 512           # 4 query-column chunks of 512

_cache = {}


def _build():
    nc = bacc.Bacc()
    # host-packed layouts (see _prep_core_inputs):
    #   xP   : 32 blocks (th, jh, k) of [128, 512]          -> [4096, 512]
    #   wqP  : 8 blocks (m) of [128, 8*128] (k-major cols)  -> [1024, 1024]
    #   wvP  : [128, 8*512] (k-major col blocks)
    #   wpP  : [128, 4*1024] bf16 (k-major col blocks)
    xP = nc.declare_dram_parameter("xP", [32 * 128, 512], f32r, isOutput=False)
    wqP = nc.declare_dram_parameter("wqP", [8 * 128, 1024], f32r, isOutput=False)
    wvP = nc.declare_dram_parameter("wvP", [128, 8 * 512], f32r, isOutput=False)
    wpP = nc.declare_dram_parameter("wpP", [128, 4 * 1024], bf16, isOutput=False)
    trimask = nc.declare_dram_parameter("trimask", [128, 128], bf16, isOutput=False)
    outp = nc.declare_dram_parameter("out", [T, D_MODEL], f32, isOutput=True)

    with TileContext(nc) as tc, ExitStack() as outer:
        qkp = outer.enter_context(tc.tile_pool(name="qk", bufs=1))
        vp = outer.enter_context(tc.tile_pool(name="v", bufs=1))
        smp = outer.enter_context(tc.tile_pool(name="small", bufs=1))
        ysbp = outer.enter_context(tc.tile_pool(name="ysb", bufs=1))
        wpp = outer.enter_context(tc.tile_pool(name="wp", bufs=1))

        qk = [qkp.tile([128, T], f32r, tag=f"qk{m}", name=f"qk{m}") for m in range(8)]
        ysb = [ysbp.tile([128, T], bf16, tag=f"y{k}", name=f"ysb{k}") for k in range(4)]
        vt = [None] * NTT
        mask = smp.tile([128, 128], bf16)
        warm = smp.tile([2, 128], bf16)
        wpt = wpp.tile([128, 4 * 1024], bf16)

        # ---- Phase A: qkT[o,t] (S1a, m-outer) + v tiles, x fully resident ----
        with ExitStack() as s1:
            xp = s1.enter_context(tc.tile_pool(name="x", bufs=1))
            wvp = s1.enter_context(tc.tile_pool(name="wv", bufs=1))
            wqp = s1.enter_context(tc.tile_pool(name="wq", bufs=2))
            ps1 = s1.enter_context(tc.tile_pool(name="ps1", bufs=4, space="PSUM"))
            psv = s1.enter_context(tc.tile_pool(name="psv", bufs=2, space="PSUM"))

            xts = {}
            for th in range(2):
                for k in range(NTK):
                    xts[(th, k)] = xp.tile([128, 1024], f32r, tag=f"x{th}_{k}",
                                           name=f"x{th}_{k}")
            # x first: S1a m=0 j=0 gates on blocks (0, 0, *) = first 2MB
            for th in range(2):
                for jh in range(2):
                    for k in range(NTK):
                        blk = th * 16 + jh * 8 + k
                        nc.scalar.dma_start(
                            out=xts[(th, k)][:, jh * 512:(jh + 1) * 512],
                            in_=xP[blk * 128:(blk + 1) * 128, :])
            wvt = wvp.tile([128, 8 * 512], f32r)
            nc.sync.dma_start(out=wvt[:], in_=wvP[:, :])
            nc.sync.dma_start(out=mask[:], in_=trimask[:, :])
            nc.sync.dma_start(out=wpt[:], in_=wpP[:, :])
            # warm up the GPSIMD custom-op library load during S1
            nc.gpsimd.partition_broadcast(warm[:], mask[0:1, :])

            # S1a: q,k transposed; weights loaded once per m
            for m in range(8):
                wqm = wqp.tile([128, 1024], f32r, tag="wq", name=f"wq{m}")
                nc.gpsimd.dma_start(out=wqm[:], in_=wqP[m * 128:(m + 1) * 128, :])
                for th in range(2):
                    for j in range(2):
                        ps = ps1.tile([128, 512], f32, tag="ps", name="ps1t")
                        for k in range(NTK):
                            nc.tensor.matmul(
                                ps[:], wqm[:, k * 128:(k + 1) * 128],
                                xts[(th, k)][:, j * 512:(j + 1) * 512],
                                start=(k == 0), stop=(k == NTK - 1))
                        nc.vector.tensor_copy(
                            qk[m][:, th * 1024 + j * 512:th * 1024 + (j + 1) * 512],
                            ps[:])

            # S1b: v natural layout + ones col, bf16
            for t in range(NTT):
                th, tl = t // 8, t % 8
                va = vp.tile([128, HPC * 65], bf16, tag=f"v{t}", name=f"v{t}")
                va3 = va[:].rearrange("p (h e) -> p h e", e=65)
                nc.vector.memset(va3[:, :, 64], 1.0)
                ps = psv.tile([128, HD], f32, tag="psv", name="psvt")
                for k in range(NTK):
                    nc.tensor.matmul(ps[:], xts[(th, k)][:, tl * 128:(tl + 1) * 128],
                                     wvt[:, k * 512:(k + 1) * 512],
                                     start=(k == 0), stop=(k == NTK - 1))
                nc.vector.tensor_copy(
                    va3[:, :, 0:64],
                    ps[:].rearrange("p (h e) -> p h e", e=64))
                vt[t] = va

        # ---- Phase B: attention at 512-col query chunks + S4 woven in ----
        with ExitStack() as sb:
            pp = sb.enter_context(tc.tile_pool(name="p", bufs=6))
            bcp = sb.enter_context(tc.tile_pool(name="bc", bufs=2))
            obp = sb.enter_context(tc.tile_pool(name="ob", bufs=2))
            psA = sb.enter_context(tc.tile_pool(name="psA", bufs=2, space="PSUM"))
            psY = sb.enter_context(tc.tile_pool(name="psY", bufs=1, space="PSUM"))
            psS = sb.enter_context(tc.tile_pool(name="psS", bufs=1, space="PSUM"))

            fillers = deque()

            def pump():
                if fillers:
                    fillers.popleft()()

            def s4_units(jc):
                # projection for t-tiles in this query chunk; all ysb ready
                units = []
                for tl in range(4):
                    t = 4 * jc + tl
                    ob = [None]

                    def alloc_ob(t=t, ob=ob):
                        ob[0] = obp.tile([128, 1024], f32, tag="o", name=f"ob{t}")

                    def mm(oc, t=t, ob=ob):
                        ps = psS.tile([128, 512], f32, tag="s4", name="ps4t")
                        for k in range(4):
                            nc.tensor.matmul(
                                ps[:], ysb[k][:, t * 128:(t + 1) * 128],
                                wpt[:, k * 1024 + oc * 512:k * 1024 + (oc + 1) * 512],
                                start=(k == 0), stop=(k == 3))
                        nc.vector.tensor_copy(ob[0][:, oc * 512:(oc + 1) * 512], ps[:])

                    def out_dma(t=t, ob=ob):
                        nc.sync.dma_start(out=outp[t * 128:(t + 1) * 128, :],
                                          in_=ob[0][:])

                    units.append(alloc_ob)
                    units.append(lambda t=t, ob=ob: mm(0, t, ob))
                    units.append(lambda t=t, ob=ob: mm(1, t, ob))
                    units.append(out_dma)
                return units

            for jc in range(NJC):
                nblk = 4 * jc + 4
                for m in range(4):
                    hA, hB = 2 * m, 2 * m + 1
                    qt, kt = qk[m], qk[4 + m]
                    psyA = psY.tile([65, 512], f32, tag="pA", name="psyAt")
                    psyB = psY.tile([65, 512], f32, tag="pB", name="psyBt")
                    for i in range(nblk):
                        su = max(0, 128 * i - 512 * jc)
                        n = 512 - su
                        qcol = 512 * jc + su
                        psa = psA.tile([128, 1024], f32, tag="psa", name="psat")
                        nc.tensor.matmul(
                            psa[:, su:512], kt[0:64, i * 128:(i + 1) * 128],
                            qt[0:64, qcol:qcol + n],
                            start=True, stop=True, tile_position=(0, 0))
                        nc.tensor.matmul(
                            psa[:, 512 + su:1024], kt[64:128, i * 128:(i + 1) * 128],
                            qt[64:128, qcol:qcol + n],
                            start=True, stop=True, tile_position=(64, 0))
                        pump()
                        pt = pp.tile([128, 1024], bf16, tag="p", name="ptile")
                        p3i = psa[:].rearrange("p (g c) -> p g c", g=2)
                        p3o = pt[:].rearrange("p (g c) -> p g c", g=2)
                        nc.scalar.activation(p3o[:, :, su:512], p3i[:, :, su:512],
                                             AF.Exp)
                        if i >= 4 * jc:  # diagonal block: triangular mask
                            nc.vector.tensor_tensor(
                                pt[:, su:su + 128], pt[:, su:su + 128],
                                mask[:], ALU.mult)
                            nc.vector.tensor_tensor(
                                pt[:, 512 + su:512 + su + 128],
                                pt[:, 512 + su:512 + su + 128],
                                mask[:], ALU.mult)
                        nc.tensor.matmul(
                            psyA[:, su:512], vt[i][:, 65 * hA:65 * hA + 65],
                            pt[:, su:512], start=(i == 0), stop=(i == nblk - 1))
                        nc.tensor.matmul(
                            psyB[:, su:512], vt[i][:, 65 * hB:65 * hB + 65],
                            pt[:, 512 + su:1024], start=(i == 0), stop=(i == nblk - 1))
                        pump()
                    # normalize: recip of ones-row, broadcast, scale into ysb
                    win = slice(512 * jc, 512 * (jc + 1))
                    for psy, rs in ((psyA, slice(0, 64)), (psyB, slice(64, 128))):
                        rc = bcp.tile([1, 512], f32, tag="rc", name="rct")
                        nc.vector.reciprocal(rc[:], psy[64:65, :])
                        bc = bcp.tile([64, 512], f32, tag="bc", name="bct")
                        nc.gpsimd.partition_broadcast(bc[:], rc[:])
                        nc.vector.tensor_tensor(
                            ysb[m][rs, win], psy[0:64, :], bc[:], ALU.mult)
                if jc < NJC - 1:
                    fillers.extend(s4_units(jc))
                else:
                    while fillers:
                        pump()
                    for u in s4_units(jc):
                        u()

    nc.compile()
    return nc


def _prep_core_inputs(x, w_qkv, w_proj, c):
    b, g = c // 2, c % 2
    scale = np.float32(D_HEAD ** -0.5)
    wq = (w_qkv[g * HD:(g + 1) * HD] * scale).astype(np.float32)
    wk = w_qkv[D_MODEL + g * HD:D_MODEL + (g + 1) * HD]
    wv = w_qkv[2 * D_MODEL + g * HD:2 * D_MODEL + (g + 1) * HD]

    # xP: blocks (th, jh, k) of [128, 512] from xT = x[b].T [1024, 2048]
    xT = np.ascontiguousarray(x[b].T)  # [1024 feat, 2048 tok]
    xb = xT.reshape(8, 128, 2, 2, 512)          # [k, p, th, jh, c]
    xP = np.ascontiguousarray(xb.transpose(2, 3, 0, 1, 4)).reshape(32 * 128, 512)

    # wqP: per m [128 feat-in-chunk, (k, 128 outs)]
    wqk = np.concatenate([wq, wk], 0)           # [1024 outs, 1024 feat]
    wqkT = wqk.T                                # [1024 feat, 1024 outs]
    wq4 = wqkT.reshape(8, 128, 8, 128)          # [k, p, m, o]
    wqP = np.ascontiguousarray(wq4.transpose(2, 1, 0, 3)).reshape(8 * 128, 8 * 128)

    # wvP: [128 feat-in-chunk, (k, 512 outs)]
    wvT = wv.T                                  # [1024 feat, 512 outs]
    wv4 = wvT.reshape(8, 128, 512)              # [k, p, o]
    wvP = np.ascontiguousarray(wv4.transpose(1, 0, 2)).reshape(128, 8 * 512)

    # wpP: [128 dim-in-chunk, (k, 1024 outs)] bf16
    wpT = np.ascontiguousarray(w_proj[:, g * HD:(g + 1) * HD].T)  # [512 dims, 1024]
    wp4 = wpT.reshape(4, 128, 1024)             # [k, p, o]
    wpP = np.ascontiguousarray(wp4.transpose(1, 0, 2)).reshape(128, 4 * 1024)

    tri = np.triu(np.ones((128, 128), dtype=np.float32))
    return {
        "xP": xP,
        "wqP": wqP,
        "wvP": wvP,
        "wpP": wpP.astype(ml_dtypes.bfloat16),
        "trimask": tri.astype(ml_dtypes.bfloat16),
    }


def kernel(x, w_qkv, w_proj):
    x = np.asarray(x)
    w_qkv = np.asarray(w_qkv)
    w_proj = np.asarray(w_proj)
    if "nc" not in _cache:
        _cache["nc"] = _build()
    nc = _cache["nc"]
    in_maps = [_prep_core_inputs(x, w_qkv, w_proj, c) for c in range(N_CORES)]
    res = run_bass_kernel_spmd(nc, in_maps, core_ids=list(range(N_CORES)))
    outs = [res.results[c]["out"] for c in range(N_CORES)]
    return np.stack([outs[2 * b] + outs[2 * b + 1] for b in range(B)], 0)


# revision 6
# speedup vs baseline: 1.1018x; 1.1018x over previous
import sys
import numpy as np
import ml_dtypes

sys.path.insert(0, '/opt/trn_rl_repo')

import concourse.bacc as bacc
import concourse.mybir as mybir
from concourse.bass_utils import run_bass_kernel_spmd
from concourse.tile import TileContext
from contextlib import ExitStack
from collections import deque

f32 = mybir.dt.float32
f32r = mybir.dt.float32r
bf16 = mybir.dt.bfloat16
AF = mybir.ActivationFunctionType
ALU = mybir.AluOpType

D_MODEL = 1024
N_HEAD = 16
D_HEAD = 64
B = 4
T = 2048
N_CORES = 8
HPC = N_HEAD // 2        # 8 heads per core
HD = HPC * D_HEAD        # 512 head-dims per core
NTK = D_MODEL // 128     # 8 k-chunks over model dim
NTT = T // 128           # 16 T-tiles of 128
NJC = T // 512           # 4 query-column chunks of 512

_cache = {}


def _build():
    nc = bacc.Bacc()
    # host-packed layouts (see _prep_core_inputs):
    #   xP   : 32 blocks (th, jh, k) of [128, 512]          -> [4096, 512]
    #   wqP  : 8 blocks (m) of [128, 8*128] (k-major cols)  -> [1024, 1024]
    #   wvP  : [128, 8*512] (k-major col blocks)
    #   wpP  : [128, 4*1024] bf16 (k-major col blocks)
    xP = nc.declare_dram_parameter("xP", [32 * 128, 512], f32r, isOutput=False)
    wqP = nc.declare_dram_parameter("wqP", [8 * 128, 1024], f32r, isOutput=False)
    wvP = nc.declare_dram_parameter("wvP", [128, 8 * 512], f32r, isOutput=False)
    wpP = nc.declare_dram_parameter("wpP", [128, 4 * 1024], bf16, isOutput=False)
    trimask = nc.declare_dram_parameter("trimask", [128, 128], bf16, isOutput=False)
    outp = nc.declare_dram_parameter("out", [T, D_MODEL], f32, isOutput=True)

    with TileContext(nc) as tc, ExitStack() as outer:
        qkp = outer.enter_context(tc.tile_pool(name="qk", bufs=1))
        vp = outer.enter_context(tc.tile_pool(name="v", bufs=1))
        smp = outer.enter_context(tc.tile_pool(name="small", bufs=1))
        wpp = outer.enter_context(tc.tile_pool(name="wp", bufs=1))

        qk = [qkp.tile([128, T], f32r, tag=f"qk{m}", name=f"qk{m}") for m in range(8)]
        vt = [None] * NTT
        mask = smp.tile([128, 128], bf16)
        warm = smp.tile([2, 128], bf16)
        wpt = wpp.tile([128, 4 * 1024], bf16)

        # ---- Phase A: qkT[o,t] (S1a) + v tiles; all weights resident ----
        with ExitStack() as s1:
            xp = s1.enter_context(tc.tile_pool(name="x", bufs=1))
            wvp = s1.enter_context(tc.tile_pool(name="wv", bufs=1))
            wqp = s1.enter_context(tc.tile_pool(name="wq", bufs=1))
            ps1 = s1.enter_context(tc.tile_pool(name="ps1", bufs=4, space="PSUM"))
            psv = s1.enter_context(tc.tile_pool(name="psv", bufs=2, space="PSUM"))

            xts = {}
            for th in range(2):
                for k in range(NTK):
                    xts[(th, k)] = xp.tile([128, 1024], f32r, tag=f"x{th}_{k}",
                                           name=f"x{th}_{k}")
            wqall = wqp.tile([128, 8 * 1024], f32r)
            # first 2MB of x + m=0 weights gate the first matmul group
            for jh in range(2):
                for k in range(NTK):
                    blk = jh * 8 + k
                    nc.sync.dma_start(
                        out=xts[(0, k)][:, jh * 512:(jh + 1) * 512],
                        in_=xP[blk * 128:(blk + 1) * 128, :])
                if jh == 0:
                    for m in range(8):
                        nc.gpsimd.dma_start(
                            out=wqall[:, m * 1024:(m + 1) * 1024],
                            in_=wqP[m * 128:(m + 1) * 128, :])
            for jh in range(2):
                for k in range(NTK):
                    blk = 16 + jh * 8 + k
                    nc.sync.dma_start(
                        out=xts[(1, k)][:, jh * 512:(jh + 1) * 512],
                        in_=xP[blk * 128:(blk + 1) * 128, :])
            wvt = wvp.tile([128, 8 * 512], f32r)
            nc.scalar.dma_start(out=wvt[:], in_=wvP[:, :])
            nc.scalar.dma_start(out=mask[:], in_=trimask[:, :])
            nc.scalar.dma_start(out=wpt[:], in_=wpP[:, :])
            # warm up the GPSIMD custom-op library load during S1
            nc.gpsimd.partition_broadcast(warm[:], mask[0:1, :])

            def emit_vtile(t):
                th, tl = t // 8, t % 8
                va = vp.tile([128, HPC * 65], bf16, tag=f"v{t}", name=f"v{t}")
                va3 = va[:].rearrange("p (h e) -> p h e", e=65)
                nc.vector.memset(va3[:, :, 64], 1.0)
                ps = psv.tile([128, HD], f32, tag="psv", name="psvt")
                for k in range(NTK):
                    nc.tensor.matmul(ps[:], xts[(th, k)][:, tl * 128:(tl + 1) * 128],
                                     wvt[:, k * 512:(k + 1) * 512],
                                     start=(k == 0), stop=(k == NTK - 1))
                nc.vector.tensor_copy(
                    va3[:, :, 0:64],
                    ps[:].rearrange("p (h e) -> p h e", e=64))
                vt[t] = va

            for th in range(2):
                for m in range(8):
                    for j in range(2):
                        ps = ps1.tile([128, 512], f32, tag="ps", name="ps1t")
                        for k in range(NTK):
                            nc.tensor.matmul(
                                ps[:], wqall[:, m * 1024 + k * 128:m * 1024 + (k + 1) * 128],
                                xts[(th, k)][:, j * 512:(j + 1) * 512],
                                start=(k == 0), stop=(k == NTK - 1))
                        nc.vector.tensor_copy(
                            qk[m][:, th * 1024 + j * 512:th * 1024 + (j + 1) * 512],
                            ps[:])
                for tl in range(8):
                    emit_vtile(8 * th + tl)

        # ---- Phase B: attention at 512-col query chunks + S4 woven in ----
        with ExitStack() as sb:
            ysbp = sb.enter_context(tc.tile_pool(name="ysb", bufs=1))
            pp = sb.enter_context(tc.tile_pool(name="p", bufs=6))
            bcp = sb.enter_context(tc.tile_pool(name="bc", bufs=2))
            obp = sb.enter_context(tc.tile_pool(name="ob", bufs=2))
            psA = sb.enter_context(tc.tile_pool(name="psA", bufs=2, space="PSUM"))
            psY = sb.enter_context(tc.tile_pool(name="psY", bufs=2, space="PSUM"))

            ysb = [ysbp.tile([128, T], bf16, tag=f"y{k}", name=f"ysb{k}")
                   for k in range(4)]
            fillers = deque()

            def pump():
                if fillers:
                    fillers.popleft()()

            def s4_units(jc):
                # projection for t-tiles in this query chunk; all ysb ready
                units = []
                for tl in range(4):
                    t = 4 * jc + tl
                    ob = [None]

                    def alloc_ob(t=t, ob=ob):
                        ob[0] = obp.tile([128, 1024], f32, tag="o", name=f"ob{t}")

                    def mm(oc, t=t, ob=ob):
                        # S4 psum shares the psA ring
                        ps = psA.tile([128, 1024], f32, tag="psa", name="ps4t")
                        for k in range(4):
                            nc.tensor.matmul(
                                ps[:, 0:512], ysb[k][:, t * 128:(t + 1) * 128],
                                wpt[:, k * 1024 + oc * 512:k * 1024 + (oc + 1) * 512],
                                start=(k == 0), stop=(k == 3))
                        nc.vector.tensor_copy(ob[0][:, oc * 512:(oc + 1) * 512],
                                              ps[:, 0:512])

                    def out_dma(t=t, ob=ob):
                        nc.sync.dma_start(out=outp[t * 128:(t + 1) * 128, :],
                                          in_=ob[0][:])

                    units.append(alloc_ob)
                    units.append(lambda t=t, ob=ob: mm(0, t, ob))
                    units.append(lambda t=t, ob=ob: mm(1, t, ob))
                    units.append(out_dma)
                return units

            for jc in range(NJC):
                nblk = 4 * jc + 4
                for m in range(4):
                    hA, hB = 2 * m, 2 * m + 1
                    qt, kt = qk[m], qk[4 + m]
                    psyA = psY.tile([65, 512], f32, tag="pA", name="psyAt")
                    psyB = psY.tile([65, 512], f32, tag="pB", name="psyBt")
                    for i in range(nblk):
                        su = max(0, 128 * i - 512 * jc)
                        n = 512 - su
                        qcol = 512 * jc + su
                        psa = psA.tile([128, 1024], f32, tag="psa", name="psat")
                        nc.tensor.matmul(
                            psa[:, su:512], kt[0:64, i * 128:(i + 1) * 128],
                            qt[0:64, qcol:qcol + n],
                            start=True, stop=True, tile_position=(0, 0))
                        nc.tensor.matmul(
                            psa[:, 512 + su:1024], kt[64:128, i * 128:(i + 1) * 128],
                            qt[64:128, qcol:qcol + n],
                            start=True, stop=True, tile_position=(64, 0))
                        pump()
                        pt = pp.tile([128, 1024], bf16, tag="p", name="ptile")
                        p3i = psa[:].rearrange("p (g c) -> p g c", g=2)
                        p3o = pt[:].rearrange("p (g c) -> p g c", g=2)
                        nc.scalar.activation(p3o[:, :, su:512], p3i[:, :, su:512],
                                             AF.Exp)
                        if i >= 4 * jc:  # diagonal block: triangular mask
                            nc.vector.tensor_tensor(
                                pt[:, su:su + 128], pt[:, su:su + 128],
                                mask[:], ALU.mult)
                            nc.vector.tensor_tensor(
                                pt[:, 512 + su:512 + su + 128],
                                pt[:, 512 + su:512 + su + 128],
                                mask[:], ALU.mult)
                        nc.tensor.matmul(
                            psyA[:, su:512], vt[i][:, 65 * hA:65 * hA + 65],
                            pt[:, su:512], start=(i == 0), stop=(i == nblk - 1))
                        nc.tensor.matmul(
                            psyB[:, su:512], vt[i][:, 65 * hB:65 * hB + 65],
                            pt[:, 512 + su:1024], start=(i == 0), stop=(i == nblk - 1))
                        pump()
                    # normalize: recip of ones-row, broadcast, scale into ysb
                    win = slice(512 * jc, 512 * (jc + 1))
                    for psy, rs in ((psyA, slice(0, 64)), (psyB, slice(64, 128))):
                        rc = bcp.tile([1, 512], f32, tag="rc", name="rct")
                        nc.vector.reciprocal(rc[:], psy[64:65, :])
                        pump()
                        bc = bcp.tile([64, 512], f32, tag="bc", name="bct")
                        nc.gpsimd.partition_broadcast(bc[:], rc[:])
                        pump()
                        nc.vector.tensor_tensor(
                            ysb[m][rs, win], psy[0:64, :], bc[:], ALU.mult)
                if jc < NJC - 1:
                    fillers.extend(s4_units(jc))
                else:
                    while fillers:
                        pump()
                    for u in s4_units(jc):
                        u()

    nc.compile()
    return nc


def _prep_core_inputs(x, w_qkv, w_proj, c):
    b, g = c // 2, c % 2
    scale = np.float32(D_HEAD ** -0.5)
    wq = (w_qkv[g * HD:(g + 1) * HD] * scale).astype(np.float32)
    wk = w_qkv[D_MODEL + g * HD:D_MODEL + (g + 1) * HD]
    wv = w_qkv[2 * D_MODEL + g * HD:2 * D_MODEL + (g + 1) * HD]

    # xP: blocks (th, jh, k) of [128, 512] from xT = x[b].T [1024, 2048]
    xT = np.ascontiguousarray(x[b].T)  # [1024 feat, 2048 tok]
    xb = xT.reshape(8, 128, 2, 2, 512)          # [k, p, th, jh, c]
    xP = np.ascontiguousarray(xb.transpose(2, 3, 0, 1, 4)).reshape(32 * 128, 512)

    # wqP: per m [128 feat-in-chunk, (k, 128 outs)]
    wqk = np.concatenate([wq, wk], 0)           # [1024 outs, 1024 feat]
    wqkT = wqk.T                                # [1024 feat, 1024 outs]
    wq4 = wqkT.reshape(8, 128, 8, 128)          # [k, p, m, o]
    wqP = np.ascontiguousarray(wq4.transpose(2, 1, 0, 3)).reshape(8 * 128, 8 * 128)

    # wvP: [128 feat-in-chunk, (k, 512 outs)]
    wvT = wv.T                                  # [1024 feat, 512 outs]
    wv4 = wvT.reshape(8, 128, 512)              # [k, p, o]
    wvP = np.ascontiguousarray(wv4.transpose(1, 0, 2)).reshape(128, 8 * 512)

    # wpP: [128 dim-in-chunk, (k, 1024 outs)] bf16
    wpT = np.ascontiguousarray(w_proj[:, g * HD:(g + 1) * HD].T)  # [512 dims, 1024]
    wp4 = wpT.reshape(4, 128, 1024)             # [k, p, o]
    wpP = np.ascontiguousarray(wp4.transpose(1, 0, 2)).reshape(128, 4 * 1024)

    tri = np.triu(np.ones((128, 128), dtype=np.float32))
    return {
        "xP": xP,
        "wqP": wqP,
        "wvP": wvP,
        "wpP": wpP.astype(ml_dtypes.bfloat16),
        "trimask": tri.astype(ml_dtypes.bfloat16),
    }


def kernel(x, w_qkv, w_proj):
    x = np.asarray(x)
    w_qkv = np.asarray(w_qkv)
    w_proj = np.asarray(w_proj)
    if "nc" not in _cache:
        _cache["nc"] = _build()
    nc = _cache["nc"]
    in_maps = [_prep_core_inputs(x, w_qkv, w_proj, c) for c in range(N_CORES)]
    res = run_bass_kernel_spmd(nc, in_maps, core_ids=list(range(N_CORES)))
    outs = [res.results[c]["out"] for c in range(N_CORES)]
    return np.stack([outs[2 * b] + outs[2 * b + 1] for b in range(B)], 0)


# revision 8
# speedup vs baseline: 1.1674x; 1.0596x over previous
import sys
import numpy as np
import ml_dtypes

sys.path.insert(0, '/opt/trn_rl_repo')

import concourse.bacc as bacc
import concourse.mybir as mybir
from concourse.bass_utils import run_bass_kernel_spmd
from concourse.tile import TileContext
from contextlib import ExitStack
from collections import deque

f32 = mybir.dt.float32
f32r = mybir.dt.float32r
bf16 = mybir.dt.bfloat16
AF = mybir.ActivationFunctionType
ALU = mybir.AluOpType

D_MODEL = 1024
N_HEAD = 16
D_HEAD = 64
B = 4
T = 2048
N_CORES = 8
HPC = N_HEAD // 2        # 8 heads per core
HD = HPC * D_HEAD        # 512 head-dims per core
NTK = D_MODEL // 128     # 8 k-chunks over model dim
NTT = T // 128           # 16 T-tiles of 128
NJC = T // 512           # 4 query-column chunks of 512

_cache = {}


def _build():
    nc = bacc.Bacc()
    # host-packed layouts (see _prep_core_inputs):
    #   xP   : 32 blocks (th, jh, k) of [128, 512]          -> [4096, 512]
    #   wqP  : 8 blocks (m) of [128, 8*128] (k-major cols)  -> [1024, 1024]
    #   wvP  : [128, 8*512] (k-major col blocks)
    #   wpP  : [128, 4*1024] bf16 (k-major col blocks)
    xP = nc.declare_dram_parameter("xP", [32 * 128, 512], f32r, isOutput=False)
    wqP = nc.declare_dram_parameter("wqP", [8 * 128, 1024], f32r, isOutput=False)
    wvP = nc.declare_dram_parameter("wvP", [128, 8 * 512], f32r, isOutput=False)
    wpP = nc.declare_dram_parameter("wpP", [128, 4 * 1024], bf16, isOutput=False)
    trimask = nc.declare_dram_parameter("trimask", [128, 128], bf16, isOutput=False)
    outp = nc.declare_dram_parameter("out", [T, D_MODEL], f32, isOutput=True)

    with TileContext(nc) as tc, ExitStack() as outer:
        qkp = outer.enter_context(tc.tile_pool(name="qk", bufs=1))
        vp = outer.enter_context(tc.tile_pool(name="v", bufs=1))
        smp = outer.enter_context(tc.tile_pool(name="small", bufs=1))
        wpp = outer.enter_context(tc.tile_pool(name="wp", bufs=1))

        qk = [qkp.tile([128, T], f32r, tag=f"qk{m}", name=f"qk{m}") for m in range(8)]
        vt = [None] * NTT
        mask = smp.tile([128, 128], bf16)
        warm = smp.tile([2, 128], bf16)
        wpt = wpp.tile([128, 4 * 1024], bf16)

        # ---- Phase A: qkT[o,t] (S1a) + v tiles; all weights resident ----
        with ExitStack() as s1:
            xp = s1.enter_context(tc.tile_pool(name="x", bufs=1))
            wvp = s1.enter_context(tc.tile_pool(name="wv", bufs=1))
            wqp = s1.enter_context(tc.tile_pool(name="wq", bufs=1))
            ps1 = s1.enter_context(tc.tile_pool(name="ps1", bufs=4, space="PSUM"))
            psv = s1.enter_context(tc.tile_pool(name="psv", bufs=2, space="PSUM"))

            xts = {}
            for th in range(2):
                for k in range(NTK):
                    xts[(th, k)] = xp.tile([128, 1024], f32r, tag=f"x{th}_{k}",
                                           name=f"x{th}_{k}")
            wqall = wqp.tile([128, 8 * 1024], f32r)
            # first 2MB of x + m=0 weights gate the first matmul group
            for jh in range(2):
                for k in range(NTK):
                    blk = jh * 8 + k
                    nc.sync.dma_start(
                        out=xts[(0, k)][:, jh * 512:(jh + 1) * 512],
                        in_=xP[blk * 128:(blk + 1) * 128, :])
                if jh == 0:
                    for m in range(8):
                        nc.gpsimd.dma_start(
                            out=wqall[:, m * 1024:(m + 1) * 1024],
                            in_=wqP[m * 128:(m + 1) * 128, :])
            for jh in range(2):
                for k in range(NTK):
                    blk = 16 + jh * 8 + k
                    nc.sync.dma_start(
                        out=xts[(1, k)][:, jh * 512:(jh + 1) * 512],
                        in_=xP[blk * 128:(blk + 1) * 128, :])
            wvt = wvp.tile([128, 8 * 512], f32r)
            nc.scalar.dma_start(out=wvt[:], in_=wvP[:, :])
            nc.scalar.dma_start(out=mask[:], in_=trimask[:, :])
            nc.scalar.dma_start(out=wpt[:], in_=wpP[:, :])
            # warm up the GPSIMD custom-op library load during S1
            nc.gpsimd.partition_broadcast(warm[:], mask[0:1, :])

            def emit_vtile(t):
                th, tl = t // 8, t % 8
                va = vp.tile([128, HPC * 65], bf16, tag=f"v{t}", name=f"v{t}")
                va3 = va[:].rearrange("p (h e) -> p h e", e=65)
                nc.vector.memset(va3[:, :, 64], 1.0)
                ps = psv.tile([128, HD], f32, tag="psv", name="psvt")
                for k in range(NTK):
                    nc.tensor.matmul(ps[:], xts[(th, k)][:, tl * 128:(tl + 1) * 128],
                                     wvt[:, k * 512:(k + 1) * 512],
                                     start=(k == 0), stop=(k == NTK - 1))
                nc.vector.tensor_copy(
                    va3[:, :, 0:64],
                    ps[:].rearrange("p (h e) -> p h e", e=64))
                vt[t] = va

            for th in range(2):
                for m in range(8):
                    for j in range(2):
                        ps = ps1.tile([128, 512], f32, tag="ps", name="ps1t")
                        for k in range(NTK):
                            nc.tensor.matmul(
                                ps[:], wqall[:, m * 1024 + k * 128:m * 1024 + (k + 1) * 128],
                                xts[(th, k)][:, j * 512:(j + 1) * 512],
                                start=(k == 0), stop=(k == NTK - 1))
                        nc.vector.tensor_copy(
                            qk[m][:, th * 1024 + j * 512:th * 1024 + (j + 1) * 512],
                            ps[:])
                for tl in range(8):
                    emit_vtile(8 * th + tl)

        # ---- Phase B: attention at 512-col query chunks + S4 woven in ----
        with ExitStack() as sb:
            ysbp = sb.enter_context(tc.tile_pool(name="ysb", bufs=1))
            pp = sb.enter_context(tc.tile_pool(name="p", bufs=6))
            bcp = sb.enter_context(tc.tile_pool(name="bc", bufs=2))
            obp = sb.enter_context(tc.tile_pool(name="ob", bufs=2))
            psA = sb.enter_context(tc.tile_pool(name="psA", bufs=2, space="PSUM"))
            psY = sb.enter_context(tc.tile_pool(name="psY", bufs=2, space="PSUM"))

            ysb = [ysbp.tile([128, T], bf16, tag=f"y{k}", name=f"ysb{k}")
                   for k in range(4)]
            fillers = deque()

            def pump():
                if fillers:
                    fillers.popleft()()

            def s4_units(jc):
                # projection for t-tiles in this query chunk; all ysb ready
                units = []
                for tl in range(4):
                    t = 4 * jc + tl
                    ob = [None]

                    def alloc_ob(t=t, ob=ob):
                        ob[0] = obp.tile([128, 1024], f32, tag="o", name=f"ob{t}")

                    def mm(oc, t=t, ob=ob):
                        # S4 psum shares the psA ring
                        ps = psA.tile([128, 1024], f32, tag="psa", name="ps4t")
                        for k in range(4):
                            nc.tensor.matmul(
                                ps[:, 0:512], ysb[k][:, t * 128:(t + 1) * 128],
                                wpt[:, k * 1024 + oc * 512:k * 1024 + (oc + 1) * 512],
                                start=(k == 0), stop=(k == 3))
                        nc.vector.tensor_copy(ob[0][:, oc * 512:(oc + 1) * 512],
                                              ps[:, 0:512])

                    def out_dma(t=t, ob=ob):
                        nc.sync.dma_start(out=outp[t * 128:(t + 1) * 128, :],
                                          in_=ob[0][:])

                    units.append(alloc_ob)
                    units.append(lambda t=t, ob=ob: mm(0, t, ob))
                    units.append(lambda t=t, ob=ob: mm(1, t, ob))
                    units.append(out_dma)
                return units

            for jc in range(NJC):
                nblk = 4 * jc + 4
                for m in range(4):
                    hA, hB = 2 * m, 2 * m + 1
                    qt, kt = qk[m], qk[4 + m]
                    psyA = psY.tile([65, 512], f32, tag="pA", name="psyAt")
                    psyB = psY.tile([65, 512], f32, tag="pB", name="psyBt")
                    for i in range(nblk):
                        su = max(0, 128 * i - 512 * jc)
                        n = 512 - su
                        qcol = 512 * jc + su
                        psa = psA.tile([128, 1024], f32, tag="psa", name="psat")
                        nc.tensor.matmul(
                            psa[:, su:512], kt[0:64, i * 128:(i + 1) * 128],
                            qt[0:64, qcol:qcol + n],
                            start=True, stop=True, tile_position=(0, 0))
                        nc.tensor.matmul(
                            psa[:, 512 + su:1024], kt[64:128, i * 128:(i + 1) * 128],
                            qt[64:128, qcol:qcol + n],
                            start=True, stop=True, tile_position=(64, 0))
                        pump()
                        pt = pp.tile([128, 1024], bf16, tag="p", name="ptile")
                        p3i = psa[:].rearrange("p (g c) -> p g c", g=2)
                        p3o = pt[:].rearrange("p (g c) -> p g c", g=2)
                        nc.scalar.activation(p3o[:, :, su:512], p3i[:, :, su:512],
                                             AF.Exp)
                        if i >= 4 * jc:  # diagonal block: triangular mask
                            nc.vector.tensor_tensor(
                                pt[:, su:su + 128], pt[:, su:su + 128],
                                mask[:], ALU.mult)
                            nc.vector.tensor_tensor(
                                pt[:, 512 + su:512 + su + 128],
                                pt[:, 512 + su:512 + su + 128],
                                mask[:], ALU.mult)
                        nc.tensor.matmul(
                            psyA[:, su:512], vt[i][:, 65 * hA:65 * hA + 65],
                            pt[:, su:512], start=(i == 0), stop=(i == nblk - 1))
                        nc.tensor.matmul(
                            psyB[:, su:512], vt[i][:, 65 * hB:65 * hB + 65],
                            pt[:, 512 + su:1024], start=(i == 0), stop=(i == nblk - 1))
                        pump()
                    # normalize: recip of ones-row, broadcast, scale into ysb
                    win = slice(512 * jc, 512 * (jc + 1))
                    for psy, rs in ((psyA, slice(0, 64)), (psyB, slice(64, 128))):
                        rc = bcp.tile([1, 512], f32, tag="rc", name="rct")
                        nc.vector.tensor_copy(rc[:], psy[64:65, :])
                        rc2 = bcp.tile([1, 512], f32, tag="rc2", name="rc2t")
                        nc.vector.reciprocal_approx_fast(rc2[:], rc[:])
                        pump()
                        bc = bcp.tile([64, 512], f32, tag="bc", name="bct")
                        nc.gpsimd.partition_broadcast(bc[:], rc2[:])
                        pump()
                        nc.vector.tensor_tensor(
                            ysb[m][rs, win], psy[0:64, :], bc[:], ALU.mult)
                if jc < NJC - 1:
                    fillers.extend(s4_units(jc))
                else:
                    while fillers:
                        pump()
                    for u in s4_units(jc):
                        u()

    nc.compile()
    return nc


def _prep_core_inputs(x, w_qkv, w_proj, c):
    b, g = c // 2, c % 2
    scale = np.float32(D_HEAD ** -0.5)
    wq = (w_qkv[g * HD:(g + 1) * HD] * scale).astype(np.float32)
    wk = w_qkv[D_MODEL + g * HD:D_MODEL + (g + 1) * HD]
    wv = w_qkv[2 * D_MODEL + g * HD:2 * D_MODEL + (g + 1) * HD]

    # xP: blocks (th, jh, k) of [128, 512] from xT = x[b].T [1024, 2048]
    xT = np.ascontiguousarray(x[b].T)  # [1024 feat, 2048 tok]
    xb = xT.reshape(8, 128, 2, 2, 512)          # [k, p, th, jh, c]
    xP = np.ascontiguousarray(xb.transpose(2, 3, 0, 1, 4)).reshape(32 * 128, 512)

    # wqP: per m [128 feat-in-chunk, (k, 128 outs)]
    wqk = np.concatenate([wq, wk], 0)           # [1024 outs, 1024 feat]
    wqkT = wqk.T                                # [1024 feat, 1024 outs]
    wq4 = wqkT.reshape(8, 128, 8, 128)          # [k, p, m, o]
    wqP = np.ascontiguousarray(wq4.transpose(2, 1, 0, 3)).reshape(8 * 128, 8 * 128)

    # wvP: [128 feat-in-chunk, (k, 512 outs)]
    wvT = wv.T                                  # [1024 feat, 512 outs]
    wv4 = wvT.reshape(8, 128, 512)              # [k, p, o]
    wvP = np.ascontiguousarray(wv4.transpose(1, 0, 2)).reshape(128, 8 * 512)

    # wpP: [128 dim-in-chunk, (k, 1024 outs)] bf16
    wpT = np.ascontiguousarray(w_proj[:, g * HD:(g + 1) * HD].T)  # [512 dims, 1024]
    wp4 = wpT.reshape(4, 128, 1024)             # [k, p, o]
    wpP = np.ascontiguousarray(wp4.transpose(1, 0, 2)).reshape(128, 4 * 1024)

    tri = np.triu(np.ones((128, 128), dtype=np.float32))
    return {
        "xP": xP,
        "wqP": wqP,
        "wvP": wvP,
        "wpP": wpP.astype(ml_dtypes.bfloat16),
        "trimask": tri.astype(ml_dtypes.bfloat16),
    }


def kernel(x, w_qkv, w_proj):
    x = np.asarray(x)
    w_qkv = np.asarray(w_qkv)
    w_proj = np.asarray(w_proj)
    if "nc" not in _cache:
        _cache["nc"] = _build()
    nc = _cache["nc"]
    in_maps = [_prep_core_inputs(x, w_qkv, w_proj, c) for c in range(N_CORES)]
    res = run_bass_kernel_spmd(nc, in_maps, core_ids=list(range(N_CORES)))
    outs = [res.results[c]["out"] for c in range(N_CORES)]
    return np.stack([outs[2 * b] + outs[2 * b + 1] for b in range(B)], 0)


# revision 12
# speedup vs baseline: 1.2335x; 1.0566x over previous
import sys
import numpy as np
import ml_dtypes

sys.path.insert(0, '/opt/trn_rl_repo')

import concourse.bacc as bacc
import concourse.mybir as mybir
from concourse.bass_utils import run_bass_kernel_spmd
from concourse.tile import TileContext
from contextlib import ExitStack
from collections import deque

f32 = mybir.dt.float32
f32r = mybir.dt.float32r
bf16 = mybir.dt.bfloat16
AF = mybir.ActivationFunctionType
ALU = mybir.AluOpType

D_MODEL = 1024
N_HEAD = 16
D_HEAD = 64
B = 4
T = 2048
N_CORES = 8
HPC = N_HEAD // 2        # 8 heads per core
HD = HPC * D_HEAD        # 512 head-dims per core
NTK = D_MODEL // 128     # 8 k-chunks over model dim
NTT = T // 128           # 16 T-tiles of 128
NJC = T // 512           # 4 query-column chunks of 512

_cache = {}


def _build():
    nc = bacc.Bacc()
    # host-packed layouts (see _prep_core_inputs):
    #   xP   : 32 blocks (th, jh, k) of [128, 512]          -> [4096, 512]
    #   wqP  : 8 blocks (m) of [128, 8*128] (k-major cols)  -> [1024, 1024]
    #   wvP  : [128, 8*512] (k-major col blocks)
    #   wpP  : [128, 4*1024] bf16 (k-major col blocks)
    xP = nc.declare_dram_parameter("xP", [32 * 128, 512], f32r, isOutput=False)
    wqP = nc.declare_dram_parameter("wqP", [8 * 128, 1024], f32r, isOutput=False)
    wvP = nc.declare_dram_parameter("wvP", [128, 8 * 512], f32r, isOutput=False)
    wpP = nc.declare_dram_parameter("wpP", [128, 4 * 1024], bf16, isOutput=False)
    trimask = nc.declare_dram_parameter("trimask", [128, 128], bf16, isOutput=False)
    outp = nc.declare_dram_parameter("out", [T, D_MODEL], f32, isOutput=True)

    with TileContext(nc) as tc, ExitStack() as outer:
        qkp = outer.enter_context(tc.tile_pool(name="qk", bufs=1))
        vp = outer.enter_context(tc.tile_pool(name="v", bufs=1))
        smp = outer.enter_context(tc.tile_pool(name="small", bufs=1))
        wpp = outer.enter_context(tc.tile_pool(name="wp", bufs=1))

        qk = [qkp.tile([128, T], f32r, tag=f"qk{m}", name=f"qk{m}") for m in range(8)]
        vt = [None] * NTT
        mask = smp.tile([128, 128], bf16)
        warm = smp.tile([2, 128], bf16)
        wpt = wpp.tile([128, 4 * 1024], bf16)

        # ---- Phase A: qkT[o,t] (S1a) + v tiles; all weights resident ----
        with ExitStack() as s1:
            xp = s1.enter_context(tc.tile_pool(name="x", bufs=1))
            wvp = s1.enter_context(tc.tile_pool(name="wv", bufs=1))
            wqp = s1.enter_context(tc.tile_pool(name="wq", bufs=1))
            ps1 = s1.enter_context(tc.tile_pool(name="ps1", bufs=4, space="PSUM"))
            psv = s1.enter_context(tc.tile_pool(name="psv", bufs=2, space="PSUM"))

            xts = {}
            for th in range(2):
                for jh in range(2):
                    for k in range(NTK):
                        xts[(th, jh, k)] = xp.tile(
                            [128, 512], f32r, tag=f"x{th}_{jh}_{k}",
                            name=f"x{th}_{jh}_{k}")
            wqm = [wqp.tile([128, 1024], f32r, tag=f"wq{m}", name=f"wq{m}")
                   for m in range(8)]
            # first 2MB of x + m=0 weights gate the first matmul group
            for jh in range(2):
                for k in range(NTK):
                    blk = jh * 8 + k
                    nc.sync.dma_start(
                        out=xts[(0, jh, k)][:], in_=xP[blk * 128:(blk + 1) * 128, :])
                if jh == 0:
                    for m in range(8):
                        nc.gpsimd.dma_start(
                            out=wqm[m][:], in_=wqP[m * 128:(m + 1) * 128, :])
            for jh in range(2):
                for k in range(NTK):
                    blk = 16 + jh * 8 + k
                    nc.sync.dma_start(
                        out=xts[(1, jh, k)][:], in_=xP[blk * 128:(blk + 1) * 128, :])
            wvt = wvp.tile([128, 8 * 512], f32r)
            nc.scalar.dma_start(out=wvt[:], in_=wvP[:, :])
            nc.scalar.dma_start(out=mask[:], in_=trimask[:, :])
            nc.scalar.dma_start(out=wpt[:], in_=wpP[:, :])
            # warm up the GPSIMD custom-op library load during S1
            nc.gpsimd.partition_broadcast(warm[:], mask[0:1, :])

            def emit_vtile(t):
                th, tl = t // 8, t % 8
                jh, tc = tl // 4, tl % 4
                va = vp.tile([128, HPC * 65], bf16, tag=f"v{t}", name=f"v{t}")
                va3 = va[:].rearrange("p (h e) -> p h e", e=65)
                nc.vector.memset(va3[:, :, 64], 1.0)
                ps = psv.tile([128, HD], f32, tag="psv", name="psvt")
                for k in range(NTK):
                    nc.tensor.matmul(
                        ps[:], xts[(th, jh, k)][:, tc * 128:(tc + 1) * 128],
                        wvt[:, k * 512:(k + 1) * 512],
                        start=(k == 0), stop=(k == NTK - 1))
                nc.vector.tensor_copy(
                    va3[:, :, 0:64],
                    ps[:].rearrange("p (h e) -> p h e", e=64))
                vt[t] = va

            for th in range(2):
                for j in range(2):
                    for m in range(8):
                        ps = ps1.tile([128, 512], f32, tag="ps", name="ps1t")
                        for k in range(NTK):
                            nc.tensor.matmul(
                                ps[:], wqm[m][:, k * 128:(k + 1) * 128],
                                xts[(th, j, k)][:],
                                start=(k == 0), stop=(k == NTK - 1))
                        nc.vector.tensor_copy(
                            qk[m][:, th * 1024 + j * 512:th * 1024 + (j + 1) * 512],
                            ps[:])
                    if j == 0:
                        # v tiles for the j=0 half while the j=1 x half lands
                        for tl in range(4):
                            emit_vtile(8 * th + tl)
                for tl in range(4, 8):
                    emit_vtile(8 * th + tl)

        # ---- Phase B: attention at 512-col query chunks + S4 woven in ----
        with ExitStack() as sb:
            ysbp = sb.enter_context(tc.tile_pool(name="ysb", bufs=1))
            pp = sb.enter_context(tc.tile_pool(name="p", bufs=6))
            bcp = sb.enter_context(tc.tile_pool(name="bc", bufs=2))
            obp = sb.enter_context(tc.tile_pool(name="ob", bufs=2))
            psA = sb.enter_context(tc.tile_pool(name="psA", bufs=2, space="PSUM"))
            psY = sb.enter_context(tc.tile_pool(name="psY", bufs=2, space="PSUM"))

            ysb = [ysbp.tile([128, T], bf16, tag=f"y{k}", name=f"ysb{k}")
                   for k in range(4)]
            fillers = deque()

            def pump():
                if fillers:
                    fillers.popleft()()

            def s4_units(jc):
                # projection for t-tiles in this query chunk; all ysb ready
                units = []
                for tl in range(4):
                    t = 4 * jc + tl
                    ob = [None]

                    def alloc_ob(t=t, ob=ob):
                        ob[0] = obp.tile([128, 1024], f32, tag="o", name=f"ob{t}")

                    def mm(oc, t=t, ob=ob):
                        # S4 psum shares the psA ring
                        ps = psA.tile([128, 1024], f32, tag="psa", name="ps4t")
                        for k in range(4):
                            nc.tensor.matmul(
                                ps[:, 0:512], ysb[k][:, t * 128:(t + 1) * 128],
                                wpt[:, k * 1024 + oc * 512:k * 1024 + (oc + 1) * 512],
                                start=(k == 0), stop=(k == 3))
                        nc.vector.tensor_copy(ob[0][:, oc * 512:(oc + 1) * 512],
                                              ps[:, 0:512])

                    def out_dma(t=t, ob=ob):
                        nc.sync.dma_start(out=outp[t * 128:(t + 1) * 128, :],
                                          in_=ob[0][:])

                    units.append(alloc_ob)
                    units.append(lambda t=t, ob=ob: mm(0, t, ob))
                    units.append(lambda t=t, ob=ob: mm(1, t, ob))
                    units.append(out_dma)
                return units

            for jc in range(NJC):
                nblk = 4 * jc + 4
                for m in range(4):
                    hA, hB = 2 * m, 2 * m + 1
                    qt, kt = qk[m], qk[4 + m]
                    psyA = psY.tile([65, 512], f32, tag="pA", name="psyAt")
                    psyB = psY.tile([65, 512], f32, tag="pB", name="psyBt")
                    pend = {}

                    def emit_av(i):
                        su, pt = pend.pop(i)
                        nc.tensor.matmul(
                            psyA[:, su:512], vt[i][:, 65 * hA:65 * hA + 65],
                            pt[:, su:512], start=(i == 0), stop=(i == nblk - 1))
                        nc.tensor.matmul(
                            psyB[:, su:512], vt[i][:, 65 * hB:65 * hB + 65],
                            pt[:, 512 + su:1024], start=(i == 0), stop=(i == nblk - 1))

                    for i in range(nblk):
                        su = max(0, 128 * i - 512 * jc)
                        n = 512 - su
                        qcol = 512 * jc + su
                        psa = psA.tile([128, 1024], f32, tag="psa", name="psat")
                        nc.tensor.matmul(
                            psa[:, su:512], kt[0:64, i * 128:(i + 1) * 128],
                            qt[0:64, qcol:qcol + n],
                            start=True, stop=True, tile_position=(0, 0))
                        nc.tensor.matmul(
                            psa[:, 512 + su:1024], kt[64:128, i * 128:(i + 1) * 128],
                            qt[64:128, qcol:qcol + n],
                            start=True, stop=True, tile_position=(64, 0))
                        pump()
                        pt = pp.tile([128, 1024], bf16, tag="p", name="ptile")
                        p3i = psa[:].rearrange("p (g c) -> p g c", g=2)
                        p3o = pt[:].rearrange("p (g c) -> p g c", g=2)
                        nc.scalar.activation(p3o[:, :, su:512], p3i[:, :, su:512],
                                             AF.Exp)
                        if i >= 4 * jc:  # diagonal block: triangular mask
                            nc.vector.tensor_tensor(
                                pt[:, su:su + 128], pt[:, su:su + 128],
                                mask[:], ALU.mult)
                            nc.vector.tensor_tensor(
                                pt[:, 512 + su:512 + su + 128],
                                pt[:, 512 + su:512 + su + 128],
                                mask[:], ALU.mult)
                        pend[i] = (su, pt)
                        # AV lags QK by 2 blocks so it never waits on its exp
                        if i >= 2:
                            emit_av(i - 2)
                            pump()
                    for i in (nblk - 2, nblk - 1):
                        if i in pend:
                            emit_av(i)
                            pump()
                    # normalize: recip of ones-row, broadcast, scale into ysb
                    win = slice(512 * jc, 512 * (jc + 1))
                    for psy, rs in ((psyA, slice(0, 64)), (psyB, slice(64, 128))):
                        rc = bcp.tile([1, 512], f32, tag="rc", name="rct")
                        nc.vector.tensor_copy(rc[:], psy[64:65, :])
                        rc2 = bcp.tile([1, 512], f32, tag="rc2", name="rc2t")
                        nc.vector.reciprocal_approx_fast(rc2[:], rc[:])
                        pump()
                        bc = bcp.tile([64, 512], f32, tag="bc", name="bct")
                        nc.gpsimd.partition_broadcast(bc[:], rc2[:])
                        pump()
                        nc.vector.tensor_tensor(
                            ysb[m][rs, win], psy[0:64, :], bc[:], ALU.mult)
                if jc < NJC - 1:
                    fillers.extend(s4_units(jc))
                else:
                    while fillers:
                        pump()
                    for u in s4_units(jc):
                        u()

    nc.compile()
    return nc


def _prep_core_inputs(x, w_qkv, w_proj, c):
    b, g = c // 2, c % 2
    scale = np.float32(D_HEAD ** -0.5)
    wq = (w_qkv[g * HD:(g + 1) * HD] * scale).astype(np.float32)
    wk = w_qkv[D_MODEL + g * HD:D_MODEL + (g + 1) * HD]
    wv = w_qkv[2 * D_MODEL + g * HD:2 * D_MODEL + (g + 1) * HD]

    # xP: blocks (th, jh, k) of [128, 512] from xT = x[b].T [1024, 2048]
    xT = np.ascontiguousarray(x[b].T)  # [1024 feat, 2048 tok]
    xb = xT.reshape(8, 128, 2, 2, 512)          # [k, p, th, jh, c]
    xP = np.ascontiguousarray(xb.transpose(2, 3, 0, 1, 4)).reshape(32 * 128, 512)

    # wqP: per m [128 feat-in-chunk, (k, 128 outs)]
    wqk = np.concatenate([wq, wk], 0)           # [1024 outs, 1024 feat]
    wqkT = wqk.T                                # [1024 feat, 1024 outs]
    wq4 = wqkT.reshape(8, 128, 8, 128)          # [k, p, m, o]
    wqP = np.ascontiguousarray(wq4.transpose(2, 1, 0, 3)).reshape(8 * 128, 8 * 128)

    # wvP: [128 feat-in-chunk, (k, 512 outs)]
    wvT = wv.T                                  # [1024 feat, 512 outs]
    wv4 = wvT.reshape(8, 128, 512)              # [k, p, o]
    wvP = np.ascontiguousarray(wv4.transpose(1, 0, 2)).reshape(128, 8 * 512)

    # wpP: [128 dim-in-chunk, (k, 1024 outs)] bf16
    wpT = np.ascontiguousarray(w_proj[:, g * HD:(g + 1) * HD].T)  # [512 dims, 1024]
    wp4 = wpT.reshape(4, 128, 1024)             # [k, p, o]
    wpP = np.ascontiguousarray(wp4.transpose(1, 0, 2)).reshape(128, 4 * 1024)

    tri = np.triu(np.ones((128, 128), dtype=np.float32))
    return {
        "xP": xP,
        "wqP": wqP,
        "wvP": wvP,
        "wpP": wpP.astype(ml_dtypes.bfloat16),
        "trimask": tri.astype(ml_dtypes.bfloat16),
    }


def kernel(x, w_qkv, w_proj):
    x = np.asarray(x)
    w_qkv = np.asarray(w_qkv)
    w_proj = np.asarray(w_proj)
    if "nc" not in _cache:
        _cache["nc"] = _build()
    nc = _cache["nc"]
    in_maps = [_prep_core_inputs(x, w_qkv, w_proj, c) for c in range(N_CORES)]
    res = run_bass_kernel_spmd(nc, in_maps, core_ids=list(range(N_CORES)))
    outs = [res.results[c]["out"] for c in range(N_CORES)]
    return np.stack([outs[2 * b] + outs[2 * b + 1] for b in range(B)], 0)
